# revision 12
# baseline (speedup 1.0000x reference)
"""KimiLinear KDA decode step — Trainium2 Bass kernel (8 NeuronCores).

Problem: B=128 decode batch, HK=HV=32 heads, D=128 head dim, K=4 causal conv.
  1. per-channel causal conv1d update + silu over mixed_qkv (12288 channels)
  2. split q/k/v, l2norm(q)*D^-0.5, l2norm(k)
  3. fused KDA gate g = -exp(A_log)*softplus(forget_gate + dt_bias), b=sigmoid(beta)
  4. gated delta-rule readout folded into ONE query vector:
       o = (q_hat*eg - qk*b*(k_hat*eg)) @ S + (qk*b)*v    (eg = exp(g))

Sharding: data-parallel over batch — 16 batches per core, zero cross-core
communication.  Within a core the 16 batches form 4 "chunks" of 4 batches x
32 heads = 128 (b,h) pairs.

The readout o[v] = sum_k mg[k]*S[k,v] for 512 independent (b,h) pairs is
split across BOTH compute engines so it hides under the fp16 S stream
(~19 MB/core total HBM traffic):
  - PE path (chunks 0-1): S staged [k, bh, v]; one 128x128 fp16 stationary
    matmul per (b,h) with the folded query as the single moving column.
    Per-MM cost is the array drain latency (~170ns); 256 MMs ~= 44us.
  - DVE path (chunks 2-3): S staged [bh(partitions), v, k]; one
    tensor_tensor multiply against the broadcast mg row + a pairwise
    in-place add-tree over k.  (Measured: DVE runs 1x only — no 16-bit
    packing on this HW — so ~0.54ns/free-elem; ~10us per 2MB sub-chunk.)
The preamble (conv/norms/gates) is bh-major, so reductions are free-axis
tensor_reduce and per-(b,h) scalars are native [P,1] broadcasts; the PE path
gets its k-major query/correction vectors via two 128x128 PE transposes per
chunk.  S is quantized host-side to a single fp16 copy (~5e-4 rel err,
halving the dominant stream vs fp32).
"""

import numpy as np
from ml_dtypes import bfloat16 as BF16NP

import concourse.bass as bass
import concourse.bacc as bacc
import concourse.mybir as mybir
from concourse.tile import TileContext
from concourse.bass_utils import run_bass_kernel_spmd

F32 = mybir.dt.float32
F16 = mybir.dt.bfloat16  # bf16: DVE 2x packing + PE FWL are bf16-wired
AF = mybir.ActivationFunctionType
OP = mybir.AluOpType

NCORES = 8
B, HK, HV, D, CK = 128, 32, 32, 128, 4
SEC = 3                      # q | k | v sections
BC = B // NCORES             # batches per core = 16
NC_CH = 4                    # chunks per core (4 batches x 32 heads = 128 bh)
PE_CH = 2                    # chunks handled by the tensor engine
VS = 2                       # v-split per DVE chunk
VH = D // VS                 # 64 v rows per DVE sub-chunk
NSUB = (NC_CH - PE_CH) * VS  # 4 DVE sub-chunks
NHALF = PE_CH * 2            # 4 PE half-chunks (64 bh columns each)
QKV = (2 * HK + HV) * D      # 12288

_CACHE = {}


def _build_nc():
    nc = bacc.Bacc("TRN2", target_bir_lowering=False, debug=False)
    # DVE S stream: [sub-chunk, bh, v-half, k] fp16, contiguous per sub-chunk
    s_dve = nc.declare_dram_parameter("s_dve", [NSUB, D, VH, D], F16, isOutput=False)
    # PE S stream: [half-chunk, k, bh(64), v] fp16, contiguous per half-chunk
    s_pe = nc.declare_dram_parameter("s_pe", [NHALF, D, 64 * D], F16, isOutput=False)
    cst = nc.declare_dram_parameter("cst", [D, (CK - 1) * NC_CH * SEC * D], F16,
                                    isOutput=False)
    xq = nc.declare_dram_parameter("xq", [D, NC_CH * SEC * D], F16, isOutput=False)
    cw = nc.declare_dram_parameter("cw", [D, CK * SEC * D], F16, isOutput=False)
    fgx = nc.declare_dram_parameter("fgx", [D, NC_CH * D], F32, isOutput=False)
    dtb = nc.declare_dram_parameter("dtb", [D, D], F32, isOutput=False)
    misc = nc.declare_dram_parameter("misc", [D, 8], F32, isOutput=False)
    ident = nc.declare_dram_parameter("ident", [D, D], F16, isOutput=False)
    # outputs: DVE part [sub, bh, v-half]; PE part [half, v, bh-col]
    o_dve = nc.declare_dram_parameter("o_dve", [NSUB, D, VH], F32, isOutput=True)
    o_pe = nc.declare_dram_parameter("o_pe", [NHALF, D, 64], F32, isOutput=True)

    CSD = NC_CH * SEC * D    # 1536

    with TileContext(nc) as tc:
        with (
            tc.tile_pool(name="const", bufs=1) as const,
            tc.tile_pool(name="work", bufs=1) as work,
            tc.tile_pool(name="sdve", bufs=3) as sdve,
            tc.tile_pool(name="spe", bufs=3) as spe,
            tc.tile_pool(name="pst", bufs=2, space="PSUM") as pst,
            tc.tile_pool(name="psm", bufs=2, space="PSUM") as psm,
        ):
            # ---- input staging (scalar ring; S stream interleaved on sync) -
            t_cst = const.tile([D, (CK - 1) * CSD], F16)
            nc.sync.dma_start(t_cst[:], cst[:])
            t_xq = const.tile([D, CSD], F16)
            nc.sync.dma_start(t_xq[:], xq[:])
            t_cw = const.tile([D, CK * SEC * D], F16)
            nc.sync.dma_start(t_cw[:], cw[:])
            cw_v = t_cw[:].rearrange("p (t s d) -> p t s d", t=CK, s=SEC)
            t_fg = const.tile([D, NC_CH * D], F32)
            nc.sync.dma_start(t_fg[:], fgx[:])
            fg_v = t_fg[:].rearrange("p (c d) -> p c d", c=NC_CH)
            t_dtb = const.tile([D, D], F32)
            nc.sync.dma_start(t_dtb[:], dtb[:])
            t_misc = const.tile([D, 8], F32)
            nc.sync.dma_start(t_misc[:], misc[:])
            t_id = const.tile([D, D], F16)
            nc.sync.dma_start(t_id[:], ident[:])

            # S stream: alternate DVE sub-chunks and PE half-chunks (2.1 MB
            # each) so both engines consume the stream concurrently.
            s_tiles = []
            for i in range(NSUB):
                Sd = sdve.tile([D, VH, D], F16, name=f"Sd{i}", tag="Sd")
                nc.sync.dma_start(Sd[:], s_dve[i])
                Sp = spe.tile([D, 64, D], F16, name=f"Sp{i}", tag="Sp")
                nc.sync.dma_start(Sp[:], s_pe[i])
                s_tiles.append((Sd, Sp))

            # ---- causal conv1d single-step + silu -------------------------
            cst_v = t_cst[:].rearrange("p (t f) -> p t f", t=CK - 1)
            cst3 = cst_v.rearrange("p t (c s d) -> p t c s d", c=NC_CH, s=SEC)
            acc = work.tile([D, CSD], F16)
            tmp = work.tile([D, CSD], F16)
            acc_v = acc[:].rearrange("p (c s d) -> p c s d", c=NC_CH, s=SEC)
            tmp_v = tmp[:].rearrange("p (c s d) -> p c s d", c=NC_CH, s=SEC)
            xq_v = t_xq[:].rearrange("p (c s d) -> p c s d", c=NC_CH, s=SEC)

            def wb(j):
                return cw_v[:, j, None, :, :].to_broadcast((D, NC_CH, SEC, D))

            nc.vector.tensor_tensor(acc_v[:], cst3[:, 0], wb(0), OP.mult)
            for j in (1, 2):
                nc.vector.tensor_tensor(tmp_v[:], cst3[:, j], wb(j), OP.mult)
                nc.vector.tensor_tensor(acc[:], acc[:], tmp[:], OP.add)
            nc.vector.tensor_tensor(tmp_v[:], xq_v[:], wb(3), OP.mult)
            nc.vector.tensor_tensor(acc[:], acc[:], tmp[:], OP.add)

            x = work.tile([D, NC_CH, SEC, D], F16)
            nc.scalar.activation(x[:], acc_v[:], AF.Silu)
            xqs = x[:, :, 0, :]
            xks = x[:, :, 1, :]
            xvs = x[:, :, 2, :]

            # gate input (independent of conv; Abs groups with the silu table)
            g1 = work.tile([D, NC_CH, D], F32)
            nc.vector.tensor_tensor(
                g1[:], fg_v[:], t_dtb[:, None, :].to_broadcast((D, NC_CH, D)),
                OP.add)
            ga = work.tile([D, NC_CH, D], F32)
            nc.scalar.activation(ga[:], g1[:], AF.Abs)

            # ---- l2 norms --------------------------------------------------
            sq = work.tile([D, NC_CH, D], F16)
            nq = work.tile([D, NC_CH], F32)
            nk = work.tile([D, NC_CH], F32)
            nc.vector.tensor_tensor(sq[:], xqs, xqs, OP.mult)
            nc.vector.tensor_reduce(nq[:], sq[:], mybir.AxisListType.X, OP.add)
            nc.vector.tensor_tensor(sq[:], xks, xks, OP.mult)
            nc.vector.tensor_reduce(nk[:], sq[:], mybir.AxisListType.X, OP.add)
            nc.vector.tensor_scalar_add(nq[:], nq[:], 1e-6)
            nc.vector.tensor_scalar_add(nk[:], nk[:], 1e-6)
            rq = work.tile([D, NC_CH], F32)
            rk = work.tile([D, NC_CH], F32)
            nc.vector.reciprocal(rq[:], nq[:])
            nc.vector.reciprocal(rk[:], nk[:])
            # rsqrt = sqrt(1/x); q also gets the D^-0.5 scale folded in
            nc.scalar.activation(rq[:], rq[:], AF.Sqrt, scale=1.0 / D)
            nc.scalar.activation(rk[:], rk[:], AF.Sqrt)

            # ---- KDA gate: softplus(x) = relu(x) + ln(1+exp(-|x|)) --------
            nc.scalar.activation(ga[:], ga[:], AF.Exp, scale=-1.0)
            nc.scalar.activation(ga[:], ga[:], AF.Ln, bias=1.0)
            nc.vector.tensor_scalar_max(g1[:], g1[:], 0.0)
            nc.vector.tensor_tensor(g1[:], g1[:], ga[:], OP.add)
            nc.vector.tensor_scalar(g1[:], g1[:], t_misc[:, 0:1], None, OP.mult)
            eg = work.tile([D, NC_CH, D], F16)
            nc.scalar.activation(eg[:], g1[:], AF.Exp)
            # b = sigmoid(beta) = 1/(1+exp(-beta))
            bsig = work.tile([D, NC_CH], F32)
            nc.scalar.activation(bsig[:], t_misc[:, 1:1 + NC_CH], AF.Exp,
                                 scale=-1.0)
            nc.vector.tensor_scalar_add(bsig[:], bsig[:], 1.0)
            nc.vector.reciprocal(bsig[:], bsig[:])

            # ---- fold per-(b,h) scalars -----------------------------------
            qkr = work.tile([D, NC_CH], F32)
            nc.vector.tensor_tensor(sq[:], xqs, xks, OP.mult)
            nc.vector.tensor_reduce(qkr[:], sq[:], mybir.AxisListType.X, OP.add)
            nc.vector.tensor_tensor(qkr[:], qkr[:], rq[:], OP.mult)
            nc.vector.tensor_tensor(qkr[:], qkr[:], rk[:], OP.mult)
            cvb = work.tile([D, NC_CH], F32)      # qk*b      (for the +v term)
            nc.vector.tensor_tensor(cvb[:], qkr[:], bsig[:], OP.mult)
            mgs = work.tile([D, NC_CH], F32)      # -qk*b*rk  (fold into k)
            nc.vector.tensor_tensor(mgs[:], cvb[:], rk[:], OP.mult)
            nc.vector.tensor_scalar(mgs[:], mgs[:], -1.0, None, OP.mult)

            # ---- mg = (q*rq - qk*b*rk*k) * eg  -----------------------------
            qh = work.tile([D, NC_CH, D], F16)
            mg = work.tile([D, NC_CH, D], F16)
            for c in range(NC_CH):
                nc.vector.tensor_scalar(qh[:, c, :], xqs[:, c, :],
                                        rq[:, c:c + 1], None, OP.mult)
                nc.vector.scalar_tensor_tensor(
                    mg[:, c, :], xks[:, c, :], mgs[:, c:c + 1], qh[:, c, :],
                    OP.mult, OP.add)
            nc.vector.tensor_tensor(mg[:], mg[:], eg[:], OP.mult)

            # ---- PE-chunk prep: transpose mg and cvb*v to k/v-major --------
            mgT = []
            cvvT = []
            for c in range(PE_CH):
                tp = pst.tile([D, D], F16, name=f"tp{c}", tag="tp")
                nc.tensor.transpose(tp[:], mg[:, c, :], t_id[:])
                m16 = work.tile([D, D], F16, name=f"mgT{c}", tag="mgT")
                nc.vector.tensor_copy(m16[:], tp[:])
                mgT.append(m16)
                cvv = work.tile([D, D], F16, name=f"cvv{c}", tag="cvv")
                nc.vector.tensor_scalar(cvv[:], xvs[:, c, :], cvb[:, c:c + 1],
                                        None, OP.mult)
                tp2 = pst.tile([D, D], F16, name=f"tq{c}", tag="tp")
                nc.tensor.transpose(tp2[:], cvv[:], t_id[:])
                c32 = work.tile([D, D], F32, name=f"cvvT{c}", tag="cvvT")
                nc.vector.tensor_copy(c32[:], tp2[:])
                cvvT.append(c32)

            # ---- main loop: both engines stream their S halves -------------
            P = work.tile([D, VH, D], F16)
            for i in range(NSUB):
                Sd, Sp = s_tiles[i]
                # --- PE half-chunk i: 64 per-(b,h) stationary matmuls ------
                c, hf = divmod(i, 2)
                pso = psm.tile([D, 64], F32, name=f"pso{i}", tag="pso")
                for j in range(64):
                    col = hf * 64 + j
                    nc.tensor.matmul(pso[:, j:j + 1], Sp[:, j, :],
                                     mgT[c][:, col:col + 1],
                                     start=True, stop=True)
                ope = work.tile([D, 64], F32, name=f"ope{i}", tag="ope")
                nc.vector.tensor_tensor(
                    ope[:], pso[:], cvvT[c][:, hf * 64:hf * 64 + 64], OP.add)
                nc.scalar.dma_start(o_pe[i], ope[:])

                # --- DVE sub-chunk i: broadcast multiply + add-tree --------
                cd, vh = divmod(i, VS)
                cd += PE_CH
                nc.vector.tensor_tensor(
                    P[:], Sd[:],
                    mg[:, cd, None, :].to_broadcast((D, VH, D)), OP.mult)
                w = D // 2
                while w >= 2:
                    nc.vector.tensor_tensor(
                        P[:, :, 0:w], P[:, :, 0:w], P[:, :, w:2 * w], OP.add)
                    w //= 2
                od = work.tile([D, VH], F32, name=f"od{i}", tag="od")
                nc.vector.tensor_tensor(od[:], P[:, :, 0], P[:, :, 1], OP.add)
                nc.vector.scalar_tensor_tensor(
                    od[:], xvs[:, cd, vh * VH:(vh + 1) * VH], cvb[:, cd:cd + 1],
                    od[:], OP.mult, OP.add)
                nc.scalar.dma_start(o_dve[i], od[:])

    nc.compile()
    return nc


def _prep_inputs(mixed_qkv, forget_gate, beta, conv_state, conv_weights,
                 ssm_state, A_log, dt_bias):
    mixed_qkv = np.asarray(mixed_qkv, np.float32)
    forget_gate = np.asarray(forget_gate, np.float32)
    beta = np.asarray(beta, np.float32)
    conv_state = np.asarray(conv_state, np.float32)
    conv_weights = np.asarray(conv_weights, np.float32)
    ssm_state = np.asarray(ssm_state, np.float32)
    A_log = np.asarray(A_log, np.float32)
    dt_bias = np.asarray(dt_bias, np.float32)

    # shared (weight-like) tensors
    w = conv_weights.reshape(SEC, HV, D, CK).transpose(1, 3, 0, 2)
    cw = np.broadcast_to(w[None], (4, HV, CK, SEC, D)).reshape(D, CK * SEC * D)
    cw = np.ascontiguousarray(cw).astype(BF16NP)
    dtbp = np.ascontiguousarray(
        np.broadcast_to(dt_bias.reshape(HV, D)[None], (4, HV, D)).reshape(D, D))
    nega = np.broadcast_to((-np.exp(A_log))[None], (4, HV)).reshape(D)
    identity = np.eye(D).astype(BF16NP)

    in_maps = []
    for ci in range(NCORES):
        cs = slice(ci * BC, (ci + 1) * BC)
        ssm_c = ssm_state[cs]
        # PE chunks (batches 0..7): [half, k, (bh=64, v)]
        sp = ssm_c[0:8].reshape(PE_CH, 2, 2, HV, D, D)
        sp = sp.transpose(0, 1, 4, 2, 3, 5).reshape(NHALF, D, 64 * D)
        sp = np.ascontiguousarray(sp).astype(BF16NP)
        # DVE chunks (batches 8..15): [sub, bh, v-half, k]
        sd = ssm_c[8:16].reshape(NC_CH - PE_CH, 4, HV, D, D)
        sd = sd.transpose(0, 1, 2, 4, 3).reshape(NC_CH - PE_CH, D, VS, VH, D)
        sd = sd.transpose(0, 2, 1, 3, 4).reshape(NSUB, D, VH, D)
        sd = np.ascontiguousarray(sd).astype(BF16NP)
        # conv state: [16, 12288, 3] -> [p, (tap, c, sec, d)]
        cstp = conv_state[cs].reshape(NC_CH, 4, SEC, HV, D, CK - 1)
        cstp = cstp.transpose(1, 3, 5, 0, 2, 4).reshape(D, (CK - 1) * NC_CH * SEC * D)
        cstp = np.ascontiguousarray(cstp).astype(BF16NP)
        xqp = mixed_qkv[cs].reshape(NC_CH, 4, SEC, HV, D)
        xqp = xqp.transpose(1, 3, 0, 2, 4).reshape(D, NC_CH * SEC * D)
        xqp = np.ascontiguousarray(xqp).astype(BF16NP)
        fgp = forget_gate[cs].reshape(NC_CH, 4, HV, D)
        fgp = np.ascontiguousarray(
            fgp.transpose(1, 2, 0, 3).reshape(D, NC_CH * D))
        mi = np.zeros((D, 8), np.float32)
        mi[:, 0] = nega
        bet = beta[cs].reshape(NC_CH, 4, HV).transpose(1, 2, 0).reshape(D, NC_CH)
        mi[:, 1:1 + NC_CH] = bet
        in_maps.append({
            "s_dve": sd,
            "s_pe": sp,
            "cst": cstp,
            "xq": xqp,
            "cw": cw,
            "fgx": fgp,
            "dtb": dtbp,
            "misc": mi,
            "ident": identity,
        })
    return in_maps


def run(trace=False, **inputs):
    if "nc" not in _CACHE:
        _CACHE["nc"] = _build_nc()
    nc = _CACHE["nc"]
    in_maps = _prep_inputs(**inputs)
    res = run_bass_kernel_spmd(nc, in_maps, list(range(NCORES)), trace=trace)
    outs = []
    for ci in range(NCORES):
        r = res.results[ci]
        ope = np.asarray(r["o_pe"])   # [4, 128 v, 64 (b2,h)]
        ope = ope.reshape(PE_CH, 2, D, 2, HV).transpose(0, 1, 3, 4, 2)
        o_lo = ope.reshape(8, HV, D)
        odv = np.asarray(r["o_dve"])  # [4, 128 (b4,h), 64 vh]
        odv = odv.reshape(NC_CH - PE_CH, VS, 4, HV, VH).transpose(0, 2, 3, 1, 4)
        o_hi = odv.reshape(8, HV, D)
        outs.append(np.concatenate([o_lo, o_hi], axis=0))
    return np.concatenate(outs, axis=0), res


def kernel(**inputs) -> np.ndarray:
    out, _ = run(trace=False, **inputs)
    return out
